# revision 31
# baseline (speedup 1.0000x reference)
"""Trainium2 Bass kernel for CrossModalAttention.

Reference computation (per batch row b, modalities q,k in {0,1,2}):
  qp[m] = x[m] @ Wq[m] + bq[m];  kp[m] = x[m] @ Wk[m] + bk[m]
  scores[q,k] = v[q] . tanh(qp[q] + kp[k])          (k != q)
  alpha = softmax over k (2 off-diagonal entries per q)
  att[q] = sum_k alpha[q,k] * (x[k] @ Wt[q,k] + bt[q,k])
  fused  = LayerNorm(concat_m(x[m] + att[m]); gamma, beta)

Pure data parallel over the batch across 8 NeuronCores (8192 rows per
core), parameters replicated; per core, batch tiles of 128 rows,
3-stage software pipeline (load / transpose+qk+tanh+scores / rest).

Design notes (v2):
  - x^T via per-modality DMA-crossbar transposes (dma_start_transpose,
    SP queue) instead of PE transposes + PSUM evacuation.  Per-modality
    (not one whole-row transpose) so the three transposes parallelize
    across DMA engines on hardware.
  - fp8 copy of x^T in one gpsimd cast-DMA (SBUF->SBUF).
  - qk projections pair-fused: for each query modality q one PSUM bank
    [128, 512] accumulates [qp[q]+kp[k1] | qp[q]+kp[k2]] directly (Wq
    duplicated host-side so one q-side matmul covers both halves), all
    in fp8 DoubleRow.  tanh reads the bank straight from PSUM: no
    evacuation, no separate tanh-input adds.
  - all biases are rank-1 DoubleRow fp8 matmuls ([ones|zeros]
    stationary trick), 2x cheaper than bf16 rank-1s and zero work on
    the vector engines.
  - scores with signed v: v_rep[q] = [v_q | -v_q] so one accumulated
    DVE pass per bank yields d = s1 - s2 for the sigmoid directly.
  - att: per q, PSUM banks y1 (k1 side; fp8 DoubleRow for K-chunks 0,1
    and bf16 for 2,3 - Y1_MODE='half') and y2 (k2 side, bf16); combine
    attended = a1*y1 + (a2*y2 + x) as two DVE scalar_tensor_tensor ops
    with the row-sum accumulated for free.  fp8 on only half of the
    a1-weighted path keeps rel err at ~1.5e-2 (vs 1.9e-2 all-fp8 y1,
    tolerance 2e-2).
  - LN: E[x^2] via one full-width Act Square with accumulator; rstd via
    a single Heron iteration on DVE (no Sqrt table reload); normalize
    as one all-bf16 SBUF tensor_scalar hitting the DVE 4x_2p mode.
  - output stored as bf16 on the SP queue (the data only ever had bf16
    precision); host upcasts to f32.  Stores are deferred a few
    iterations because a waiting DMA blocks its queue sequencer.
  - parameters packed host-side into 4 tensors (split into a few DMAs
    each so early-tile transposes interleave with the transfers and the
    qk-phase weights land first); weight tiles are views into the packs.
    The first x loads are emitted ahead of the parameter packs.

TimelineSim (cost model v2), 64-tile shard: 465 us/core, PE-saturated
(steady state gapless: PE ~6.7 us/tile busy, matmul floor 6.4 =
15360 cyc: qk 3840 + y1-half 4608 + y2 6912).  Measured via pipelined
dispatch: best 316 us/call in quiet windows (tunnel noise spans
286..3100 us across windows); baseline kernel ~1020-1390 us/call.
"""

import json

import numpy as np

import concourse.bass as bass
import concourse.bass2jax as bass2jax
import concourse.bass_utils as bass_utils
import concourse.mybir as mybir
import concourse.tile as tile
from concourse.bass_utils import run_bass_kernel_spmd

M, E, A = 3, 512, 256
B_FULL = 65536
N_CORES = 8
BC = B_FULL // N_CORES  # 8192 rows per core
P = 128
EC = E // P  # 4 contraction chunks
LN_EPS = 1e-5

F32 = mybir.dt.float32
BF16 = mybir.dt.bfloat16
FP8 = mybir.dt.float8e4
AL = mybir.AluOpType
AF = mybir.ActivationFunctionType
DR = mybir.MatmulPerfMode.DoubleRow

# --- tuning flags -----------------------------------------------------------
# Precision mode of the a1-weighted Wt matmul (y1):
#   'fp8'  - both K-chunk pairs in fp8 DoubleRow (fastest, ~1.7e-2 rel)
#   'half' - chunks 0,1 fp8 DR, chunks 2,3 bf16 (safer, ~1.5e-2 rel)
#   'bf16' - all bf16 (safest, ~1e-2 rel)
Y1_MODE = "half"
SCORES_ON_POOL = False
SINGLE_TP = False  # 3 per-modality transposes parallelize across DMA engines on HW
# pool buffer counts (sweepable)
BUFS = dict(xb=8, xt=6, xt8=6, tth=5, tsc=4, small=8, tmp=4, att=3, sq=3,
            outp=6, tin=3, y=5)
PIPE_STAGES = 2  # 2: load | rest+1; 3: load | transpose | rest
STORE_DEFER = 3  # extra iterations before issuing each output store
PARAMS_ON_ACT = False  # param pack DMAs on Act queue instead of SP
TP_SPLIT = 1  # transpose DMAs per modality (1 or 2)
PRELOAD = 2  # x loads emitted ahead of the parameter packs

# For query modality q the two keys, in a fixed order.
K_FIRST = [1, 0, 0]
K_SECOND = [2, 2, 1]

# ---------------------------------------------------------------------------
# The walrus build in this container rejects instructions carrying more than
# one semaphore wait.  Legalize the serialized BIR: move excess waits onto
# NoOp instructions inserted just before the offender on the same engine.
# ---------------------------------------------------------------------------
_MAX_WAITS = 1
_REAL_ENGINES = {"PE", "DVE", "Activation", "Pool", "SP"}


def _legalize_waits(bir_json) -> bytes:
    d = json.loads(bir_json)
    for f in d.get("functions", []):
        for b in f.get("blocks", []):
            insts = b.get("instructions", [])
            out = []
            for inst in insts:
                si = inst.get("sync_info")
                waits = (si or {}).get("on_wait") or []
                if len(waits) > _MAX_WAITS and inst.get("engine") in _REAL_ENGINES:
                    extra = waits[: len(waits) - _MAX_WAITS]
                    si["on_wait"] = waits[len(waits) - _MAX_WAITS :]
                    for j, w in enumerate(extra):
                        out.append(
                            {
                                "debug": inst.get("debug", 0),
                                "engine": inst["engine"],
                                "ins": [],
                                "name": f"{inst['name']}-ws{j}",
                                "opcode": "NoOp",
                                "outs": [],
                                "sync_info": {"on_update": [], "on_wait": [w]},
                            }
                        )
                out.append(inst)
            b["instructions"] = out
    return json.dumps(d).encode()


_orig_compile_bir_kernel = bass_utils.compile_bir_kernel


def _patched_compile_bir_kernel(bir_json, tmpdir, neff_name="file.neff"):
    return _orig_compile_bir_kernel(_legalize_waits(bir_json), tmpdir, neff_name)


if bass_utils.compile_bir_kernel is not _patched_compile_bir_kernel:
    bass_utils.compile_bir_kernel = _patched_compile_bir_kernel
    bass2jax.compile_bir_kernel = _patched_compile_bir_kernel


def _dr(ap):
    """Rearrange a [p, 2*F] slice into DoubleRow [p, 2, F] form."""
    return ap.rearrange("p (two f) -> p two f", two=2)


def _build(bc: int, fast_gb: bool, reps: int = 1) -> bass.Bass:
    nt = bc // P
    nc = bass.Bass()

    x_d = nc.dram_tensor("x", [M, bc, E], F32, kind="ExternalInput")
    g_d = nc.dram_tensor("gamma", [M * E], F32, kind="ExternalInput")
    be_d = nc.dram_tensor("beta", [M * E], F32, kind="ExternalInput")
    # packed pre-arranged parameters (prepared host-side in kernel()):
    # pk8: per-partition concat of wqq8 (3x2048) | wk8 (3x1024) | wt18 (3x2048)
    # pkb: wt2 (3x2048 el) | wt1b (3x1024 el, 'half' mode only)
    # rows8: bqk8 (3x1024) | bt18 (3x1024) | bt28 (3x1024)
    # rowsb: v_pm (3x512 el)
    F8TOT = 3 * (EC * 2 * A) + 3 * (EC * A) + 3 * (EC * E)
    BFTOT = 3 * (EC * E) + (3 * 2 * E if Y1_MODE == "half" else 0)
    pk8_d = nc.dram_tensor("pk8", [P, F8TOT], FP8, kind="ExternalInput")
    pkb_d = nc.dram_tensor("pkb", [P, BFTOT], BF16, kind="ExternalInput")
    rows8_d = nc.dram_tensor("rows8", [1, 9 * 1024], FP8, kind="ExternalInput")
    rowsb_d = nc.dram_tensor("rowsb", [1, 3 * 2 * A], BF16, kind="ExternalInput")
    out_d = nc.dram_tensor("out", [bc, M * E], BF16, kind="ExternalOutput")

    with tile.TileContext(nc) as tc:
        with (
            tc.tile_pool(name="const", bufs=1) as cpool,
            tc.tile_pool(name="xb", bufs=BUFS["xb"]) as xbpool,
            tc.tile_pool(name="xt", bufs=BUFS["xt"]) as xtpool,
            tc.tile_pool(name="xt8", bufs=BUFS["xt8"]) as xt8pool,
            tc.tile_pool(name="tth", bufs=BUFS["tth"]) as tthpool,
            tc.tile_pool(name="tsc", bufs=BUFS["tsc"]) as tscpool,
            tc.tile_pool(name="small", bufs=BUFS["small"]) as smpool,
            tc.tile_pool(name="tmp", bufs=BUFS["tmp"]) as tmppool,
            tc.tile_pool(name="att", bufs=BUFS["att"]) as attpool,
            tc.tile_pool(name="sq", bufs=BUFS["sq"]) as sqpool,
            tc.tile_pool(name="outp", bufs=BUFS["outp"]) as outpool,
            tc.tile_pool(name="tin_ps", bufs=BUFS["tin"], space=bass.MemorySpace.PSUM) as tinps,
            tc.tile_pool(name="y_ps", bufs=BUFS["y"], space=bass.MemorySpace.PSUM) as yps,
        ):
            pools = dict(
                xb=xbpool, xt=xtpool, xt8=xt8pool, tth=tthpool, tsc=tscpool,
                sm=smpool, tmp=tmppool, att=attpool, sq=sqpool, out=outpool,
                tin=tinps, y=yps,
            )

            # ---- resident parameters (packed loads) ----
            peng = nc.scalar if PARAMS_ON_ACT else nc.sync
            # emit the first x loads ahead of the parameter packs so the
            # tile-0/1 load->transpose chain is not sequenced behind the
            # (long) parameter transfers
            preloaded = {i: _emit_load(nc, i, x_d, pools) for i in range(PRELOAD)}
            # qk-phase params (pk8, rows) first so tile 0's matmuls can
            # start while the y-phase weights (pkb) are still in flight
            pk8 = cpool.tile([P, F8TOT], FP8, name="pk8", tag="pk8")
            for lo, hi in ((0, 6144), (6144, 9216), (9216, F8TOT)):
                peng.dma_start(pk8[:, lo:hi], pk8_d[:, lo:hi])
            rows8 = cpool.tile([1, 9 * 1024], FP8, name="rows8", tag="rows8")
            peng.dma_start(rows8[:1, :], rows8_d[:, :])
            rowsb = cpool.tile([1, 3 * 2 * A], BF16, name="rowsb", tag="rowsb")
            peng.dma_start(rowsb[:1, :], rowsb_d[:, :])
            pkb = cpool.tile([P, BFTOT], BF16, name="pkb", tag="pkb")
            step = BFTOT // 3
            for j in range(3):
                lo, hi = j * step, (j + 1) * step if j < 2 else BFTOT
                peng.dma_start(pkb[:, lo:hi], pkb_d[:, lo:hi])

            o = 0
            wqq8 = [pk8[:, o + m * 2048 : o + (m + 1) * 2048] for m in range(M)]
            o += 3 * 2048
            wk8 = [pk8[:, o + m * 1024 : o + (m + 1) * 1024] for m in range(M)]
            o += 3 * 1024
            wt1 = [pk8[:, o + m * 2048 : o + (m + 1) * 2048] for m in range(M)]
            o = 0
            wt2 = [pkb[:, o + m * 2048 : o + (m + 1) * 2048] for m in range(M)]
            o += 3 * 2048
            wt1b = (
                [pkb[:, o + m * 1024 : o + (m + 1) * 1024] for m in range(M)]
                if Y1_MODE == "half"
                else None
            )
            bqk8 = [rows8[:1, m * 1024 : (m + 1) * 1024] for m in range(M)]
            bt18 = [rows8[:1, 3072 + m * 1024 : 3072 + (m + 1) * 1024] for m in range(M)]
            bt28 = [rows8[:1, 6144 + m * 1024 : 6144 + (m + 1) * 1024] for m in range(M)]
            # [ones(128) | zeros(128)] fp8 stationary for rank-1 DR biases
            ones8 = cpool.tile([1, 2 * P], FP8, name="ones8", tag="ones8")
            nc.vector.memset(ones8[:1, 0:P], 1.0)
            nc.vector.memset(ones8[:1, P : 2 * P], 0.0)
            ones_bf = cpool.tile([1, P], BF16, name="onesb", tag="onesb")
            nc.vector.memset(ones_bf[:1, :], 1.0)

            # v_pm = [v | -v] replicated across partitions via rank-1 matmul
            v_rep = []
            for q in range(M):
                ps = yps.tile([P, 2 * A], F32, name=f"vbc{q}", tag="y")
                nc.tensor.matmul(
                    ps[:, :], ones_bf[:1, :],
                    rowsb[:1, q * 2 * A : (q + 1) * 2 * A],
                )
                rep = cpool.tile([P, 2 * A], BF16, name=f"vrep{q}", tag=f"vrep{q}")
                nc.scalar.copy(rep[:, :], ps[:, :])
                v_rep.append(rep)

            # gamma/beta replicated (general path only)
            g_rep = b_rep = None
            if not fast_gb:
                ones_f32 = cpool.tile([1, P], F32, name="onesf", tag="onesf")
                nc.vector.memset(ones_f32[:1, :], 1.0)
                grow = cpool.tile([1, M * E], F32, name="grow", tag="grow")
                nc.gpsimd.dma_start(grow[:1, :], g_d[:].rearrange("(o e) -> o e", o=1))
                brow = cpool.tile([1, M * E], F32, name="brow", tag="brow")
                nc.gpsimd.dma_start(brow[:1, :], be_d[:].rearrange("(o e) -> o e", o=1))
                g_rep = cpool.tile([P, M * E], F32, name="grep", tag="grep")
                b_rep = cpool.tile([P, M * E], F32, name="brep", tag="brep")
                for src, dst in ((grow, g_rep), (brow, b_rep)):
                    for c in range(M):
                        cs = slice(c * E, (c + 1) * E)
                        ps = yps.tile([P, E], F32, name=f"gbc{c}", tag="y")
                        nc.tensor.matmul(ps[:, :], ones_f32[:1, :], src[:1, cs])
                        nc.vector.tensor_copy(dst[:, cs], ps[:, :])

            consts = dict(
                wqq8=wqq8, wk8=wk8, wt1=wt1, wt2=wt2,
                wt1b=wt1b if Y1_MODE == "half" else None,
                bqk8=bqk8, bt18=bt18, bt28=bt28, ones8=ones8,
                v_rep=v_rep, g_rep=g_rep, b_rep=b_rep,
            )

            import contextlib

            rep_ctx = tc.For_i(0, reps, 1) if reps > 1 else contextlib.nullcontext()
            with rep_ctx:
                # 3-stage software pipeline: per iteration i emit
                #   A(i+2): x load            (Pool DMA)
                #   B(i+1): transpose, fp8 cast, qk matmuls, tanh
                #   C(i):   scores, alpha, Wt matmuls, combine, LN, store
                # so no engine stream ever waits on a same-iteration long
                # dependency chain.
                stA: dict = {}
                stB: dict = {}
                pending_store: list = []
                for i in range(nt + PIPE_STAGES):
                    if i < nt:
                        stA[i] = (
                            preloaded.pop(i)
                            if i in preloaded
                            else _emit_load(nc, i, x_d, pools)
                        )
                    if PIPE_STAGES == 3 and 0 <= i - 1 < nt:
                        _emit_transpose(nc, stA[i - 1], pools)
                    b = i - (PIPE_STAGES - 1)
                    if 0 <= b < nt:
                        stB[b] = _emit_phase1(nc, stA.pop(b), consts, pools)
                    if 0 <= i - PIPE_STAGES < nt:
                        st = _emit_phase2(
                            nc, stB.pop(i - PIPE_STAGES), fast_gb, out_d,
                            consts, pools
                        )
                        # defer stores so their waits are satisfied before
                        # the SP SEQ reaches them (a waiting DMA blocks its
                        # queue's sequencer, which would stall transposes)
                        pending_store.append(st)
                        if len(pending_store) > STORE_DEFER:
                            bs0, t0 = pending_store.pop(0)
                            nc.sync.dma_start(out_d[bs0, :], t0[:, :])
                for bs0, t0 in pending_store:
                    nc.sync.dma_start(out_d[bs0, :], t0[:, :])

    return nc


def _emit_load(nc, ti, x_d, PL):
    bs = slice(ti * P, (ti + 1) * P)
    # x for all modalities, cast f32 -> bf16 in one gpsimd DMA:
    # xb[p, m*E + e] = x[m, bs.start+p, e]
    xb = PL["xb"].tile([P, M * E], BF16, name="xb", tag="xb")
    nc.gpsimd.dma_start(
        xb[:, :].rearrange("p (m e) -> p m e", m=M),
        x_d[:, bs, :].rearrange("m p e -> p m e"),
    )
    return {"bs": bs, "xb": xb}


def _emit_transpose(nc, st, PL):
    xb = st["xb"]
    # x^T per modality via DMA crossbar transpose:
    # xt[p, m*E + c*P + b] = x^T chunk: = xb[b, m*E + c*P + p]
    xt = PL["xt"].tile([P, M * E], BF16, name="xt", tag="xt")
    if SINGLE_TP:
        nc.sync.dma_start_transpose(
            xt[:, :].rearrange("p (c b) -> p c b", c=M * EC), xb[:, :]
        )
    else:
        half = E // TP_SPLIT
        for m in range(M):
            for h in range(TP_SPLIT):
                lo = m * E + h * half
                nc.sync.dma_start_transpose(
                    xt[:, lo : lo + half].rearrange(
                        "p (c b) -> p c b", c=half // P
                    ),
                    xb[:, lo : lo + half],
                )
    st["xt"] = xt
    return st


def _emit_phase1(nc, st, C, PL):
    if "xt" not in st:
        _emit_transpose(nc, st, PL)
    xb, xt = st["xb"], st["xt"]
    # fp8 copy of x^T in one gpsimd cast-DMA (SBUF -> SBUF)
    xt8 = PL["xt8"].tile([P, M * E], FP8, name="xt8", tag="xt8")
    nc.gpsimd.dma_start(xt8[:, :], xt[:, :])

    def xt_chunk(m, c):  # bf16 x^T chunk [128, 128]
        return xt[:, m * E + c * P : m * E + (c + 1) * P]

    def xt8_pair(m, c0):  # fp8 x^T chunk-pair [128, 2, 128]
        return _dr(xt8[:, m * E + c0 * P : m * E + (c0 + 2) * P])

    # pair-fused qk projections: bank q = [qp[q]+kp[k1]+b | qp[q]+kp[k2]+b]
    tin = []
    for q in range(M):
        k1, k2 = K_FIRST[q], K_SECOND[q]
        ps = PL["tin"].tile([P, 2 * A], F32, name="tin", tag="tin")
        for i, c0 in enumerate((0, 2)):
            nc.tensor.matmul(
                ps[:, :],
                xt8_pair(q, c0),
                _dr(C["wqq8"][q][:, c0 * 2 * A : (c0 + 2) * 2 * A]),
                start=(i == 0),
                stop=False,
                perf_mode=DR,
            )
        for half, k in enumerate((k1, k2)):
            hs = slice(half * A, (half + 1) * A)
            for c0 in (0, 2):
                nc.tensor.matmul(
                    ps[:, hs],
                    xt8_pair(k, c0),
                    _dr(C["wk8"][k][:, c0 * A : (c0 + 2) * A]),
                    start=False,
                    stop=False,
                    perf_mode=DR,
                )
        nc.tensor.matmul(
            ps[:, :],
            _dr(C["ones8"][:1, :]),
            _dr(C["bqk8"][q][:1, :]),
            start=False,
            stop=True,
            perf_mode=DR,
        )
        tin.append(ps)

    # tanh straight from PSUM, one Act op per bank
    tth = PL["tth"].tile([P, 2 * M * A], BF16, name="tth", tag="tth")
    for q in range(M):
        nc.scalar.activation(
            tth[:, q * 2 * A : (q + 1) * 2 * A], tin[q][:, :], AF.Tanh
        )


    # scores: v_rep[q] = [v_q | -v_q], so one accumulated pass over the
    # whole bank gives d = s1 - s2 directly
    d_t = PL["sm"].tile([P, 4], F32, name="dsc", tag="dsc")
    eng = nc.gpsimd if SCORES_ON_POOL else nc.vector
    for q in range(M):
        tsc = PL["tsc"].tile([P, 2 * A], BF16, name="tsc", tag="tsc")
        eng.scalar_tensor_tensor(
            tsc[:, :],
            tth[:, q * 2 * A : (q + 1) * 2 * A],
            1.0,
            C["v_rep"][q][:, :],
            AL.mult,
            AL.mult,
            accum_out=d_t[:, q : q + 1],
        )

    # alpha: a1 = sigmoid(s1 - s2), a2 = 1 - a1
    a1 = PL["sm"].tile([P, 4], F32, name="a1", tag="a1")
    nc.scalar.activation(a1[:, 0:M], d_t[:, 0:M], AF.Sigmoid)
    a2 = PL["sm"].tile([P, 4], F32, name="a2", tag="a2")
    nc.vector.tensor_scalar(a2[:, 0:M], a1[:, 0:M], -1.0, 1.0, AL.mult, AL.add)

    return {"bs": st["bs"], "xb": xb, "xt": xt, "xt8": xt8,
            "a1": a1, "a2": a2, "xt_chunk": xt_chunk, "xt8_pair": xt8_pair}


def _emit_phase2(nc, st, fast_gb, out_d, C, PL):
    bs, xb, a1, a2 = st["bs"], st["xb"], st["a1"], st["a2"]
    xt_chunk, xt8_pair = st["xt_chunk"], st["xt8_pair"]

    attended = PL["att"].tile([P, M * E], BF16, name="attended", tag="attended")
    rows = PL["sm"].tile([P, 4], F32, name="rows", tag="rows")
    for q in range(M):
        k1, k2 = K_FIRST[q], K_SECOND[q]
        # y1 = x[k1] @ Wt[q,k1] + bt[q,k1]
        y1 = PL["y"].tile([P, E], F32, name="y1", tag="y")
        if Y1_MODE == "fp8":
            for i, c0 in enumerate((0, 2)):
                nc.tensor.matmul(
                    y1[:, :], xt8_pair(k1, c0),
                    _dr(C["wt1"][q][:, c0 * E : (c0 + 2) * E]),
                    start=(i == 0), stop=False, perf_mode=DR,
                )
        elif Y1_MODE == "half":
            nc.tensor.matmul(
                y1[:, :], xt8_pair(k1, 0), _dr(C["wt1"][q][:, 0 : 2 * E]),
                start=True, stop=False, perf_mode=DR,
            )
            for c in (2, 3):
                nc.tensor.matmul(
                    y1[:, :], xt_chunk(k1, c),
                    C["wt1b"][q][:, (c - 2) * E : (c - 1) * E],
                    start=False, stop=False,
                )
        else:
            for c in range(EC):
                nc.tensor.matmul(
                    y1[:, :], xt_chunk(k1, c),
                    C["wt1"][q][:, c * E : (c + 1) * E],
                    start=(c == 0), stop=False,
                )
        nc.tensor.matmul(
            y1[:, :], _dr(C["ones8"][:1, :]), _dr(C["bt18"][q][:1, :]),
            start=False, stop=True, perf_mode=DR,
        )
        # y2 = x[k2] @ Wt[q,k2] + bt[q,k2]  (bf16)
        y2 = PL["y"].tile([P, E], F32, name="y2", tag="y")
        for c in range(EC):
            nc.tensor.matmul(
                y2[:, :], xt_chunk(k2, c), C["wt2"][q][:, c * E : (c + 1) * E],
                start=(c == 0), stop=False,
            )
        nc.tensor.matmul(
            y2[:, :], _dr(C["ones8"][:1, :]), _dr(C["bt28"][q][:1, :]),
            start=False, stop=True, perf_mode=DR,
        )
        # attended_q = a1*y1 + (a2*y2 + x_q), row-sum accumulated
        qs = slice(q * E, (q + 1) * E)
        tmp = PL["tmp"].tile([P, E], BF16, name="tmp", tag="tmp")
        nc.vector.scalar_tensor_tensor(
            tmp[:, :], y2[:, :], a2[:, q : q + 1], xb[:, qs], AL.mult, AL.add
        )
        nc.vector.scalar_tensor_tensor(
            attended[:, qs], y1[:, :], a1[:, q : q + 1], tmp[:, :],
            AL.mult, AL.add, accum_out=rows[:, q : q + 1],
        )

    # LayerNorm stats: mean from rows, E[x^2] from Act Square accumulators
    mu = PL["sm"].tile([P, 1], F32, name="mu", tag="mu")
    nc.vector.tensor_reduce(mu[:, :], rows[:, 0:M], mybir.AxisListType.X, AL.add)
    nc.vector.tensor_scalar(mu[:, :], mu[:, :], 1.0 / (M * E), None, AL.mult)
    ssq = PL["sm"].tile([P, 1], F32, name="ssq", tag="ssq")
    sq = PL["sq"].tile([P, M * E], BF16, name="sq", tag="sq")
    nc.scalar.activation(
        sq[:, :], attended[:, :], AF.Square, accum_out=ssq[:, 0:1]
    )
    ex2 = PL["sm"].tile([P, 1], F32, name="ex2", tag="ex2")
    nc.vector.tensor_scalar(
        ex2[:, :], ssq[:, 0:1], 1.0 / (M * E), LN_EPS, AL.mult, AL.add
    )
    mu2 = PL["sm"].tile([P, 1], F32, name="mu2", tag="mu2")
    nc.vector.tensor_mul(mu2[:, :], mu[:, :], mu[:, :])
    varp = PL["sm"].tile([P, 1], F32, name="varp", tag="varp")
    nc.vector.tensor_sub(varp[:, :], ex2[:, :], mu2[:, :])

    # rstd = 1/sqrt(varp) via Heron on DVE only (keeps Act in the
    # tanh/sigmoid/square table set)
    sd = PL["sm"].tile([P, 1], F32, name="sd0", tag="sd0")
    nc.vector.tensor_scalar(sd[:, :], varp[:, :], 0.5, 0.5, AL.mult, AL.add)
    for it in range(1):
        rc = PL["sm"].tile([P, 1], F32, name=f"rc{it}", tag=f"rc{it}")
        nc.vector.reciprocal(rc[:, :], sd[:, :])
        sn = PL["sm"].tile([P, 1], F32, name=f"sn{it}", tag=f"sn{it}")
        nc.vector.scalar_tensor_tensor(
            sn[:, :], rc[:, :], varp[:, 0:1], sd[:, :], AL.mult, AL.add
        )
        sd2 = PL["sm"].tile([P, 1], F32, name=f"sd{it+1}", tag=f"sd{it+1}")
        nc.vector.tensor_scalar(sd2[:, :], sn[:, :], 0.5, None, AL.mult)
        sd = sd2
    rstd = PL["sm"].tile([P, 1], F32, name="rstd", tag="rstd")
    nc.vector.reciprocal(rstd[:, :], sd[:, :])

    out_t = PL["out"].tile([P, M * E], BF16, name="out", tag="out")
    # (attended - mu) * rstd: all-bf16 SBUF tensor_scalar -> 4x DVE mode
    nc.vector.tensor_scalar(
        out_t[:, :], attended[:, :], mu[:, 0:1], rstd[:, 0:1],
        AL.subtract, AL.mult,
    )
    if not fast_gb:
        gz = PL["out"].tile([P, M * E], F32, name="gz", tag="gz")
        nc.vector.tensor_mul(gz[:, :], out_t[:, :], C["g_rep"][:, :])
        nc.vector.tensor_add(gz[:, :], gz[:, :], C["b_rep"][:, :])
        nc.vector.tensor_copy(out_t[:, :], gz[:, :])
    return (bs, out_t)


_PROGRAM_CACHE: dict = {}


def _get_program(bc: int, fast_gb: bool, reps: int = 1) -> bass.Bass:
    key = (bc, fast_gb, reps)
    if key not in _PROGRAM_CACHE:
        _PROGRAM_CACHE[key] = _build(bc, fast_gb, reps)
    return _PROGRAM_CACHE[key]


def _prep_shared(ins) -> dict:
    """Host-side pre-arrangement of the (small) parameters into packed
    per-dtype tensors (one DMA each)."""
    import ml_dtypes

    BFD = ml_dtypes.bfloat16
    F8D = ml_dtypes.float8_e4m3fn
    Wq, Wk, Wt = ins["Wq"], ins["Wk"], ins["Wt"]
    bq, bk, bt = ins["bq"], ins["bk"], ins["bt"]

    def perpart(a):  # [EC, P, F] -> [P, EC*F]
        return np.ascontiguousarray(np.transpose(a, (1, 0, 2)).reshape(P, -1))

    wq_c = Wq.reshape(M, EC, P, A)
    f8_parts = [perpart(np.concatenate([wq_c[m], wq_c[m]], axis=2)) for m in range(M)]
    wk_c = Wk.reshape(M, EC, P, A)
    f8_parts += [perpart(wk_c[m]) for m in range(M)]
    wt1f = np.stack([Wt[q, K_FIRST[q]] for q in range(M)]).reshape(M, EC, P, E)
    f8_parts += [perpart(wt1f[m]) for m in range(M)]
    pk8 = np.concatenate(f8_parts, axis=1).astype(F8D)

    wt2f = np.stack([Wt[q, K_SECOND[q]] for q in range(M)]).reshape(M, EC, P, E)
    bf_parts = [perpart(wt2f[m]) for m in range(M)]
    if Y1_MODE == "half":
        bf_parts += [perpart(wt1f[m, 2:4]) for m in range(M)]
    pkb = np.concatenate(bf_parts, axis=1).astype(BFD)

    # rows: [bias | zeros] halves for the rank-1 DoubleRow trick
    z_a = np.zeros((2 * A,), np.float32)
    z_e = np.zeros((E,), np.float32)
    rows = []
    for q in range(M):
        rows.append(np.concatenate([bq[q] + bk[K_FIRST[q]],
                                    bq[q] + bk[K_SECOND[q]], z_a]))
    for q in range(M):
        rows.append(np.concatenate([bt[q, K_FIRST[q]], z_e]))
    for q in range(M):
        rows.append(np.concatenate([bt[q, K_SECOND[q]], z_e]))
    rows8 = np.concatenate(rows)[None, :].astype(F8D)
    v_pm = np.concatenate(
        [np.concatenate([ins["v"][q], -ins["v"][q]]) for q in range(M)]
    )[None, :].astype(BFD)

    return {
        "pk8": pk8,
        "pkb": np.ascontiguousarray(pkb),
        "rows8": np.ascontiguousarray(rows8),
        "rowsb": np.ascontiguousarray(v_pm),
        "gamma": ins["gamma"],
        "beta": ins["beta"],
    }


def kernel(**inputs) -> np.ndarray:
    ins = {
        k: np.ascontiguousarray(np.asarray(v, dtype=np.float32))
        for k, v in inputs.items()
    }
    x = ins["x"]
    assert x.shape == (M, B_FULL, E), x.shape
    fast_gb = bool(np.all(ins["gamma"] == 1.0) and np.all(ins["beta"] == 0.0))
    nc = _get_program(BC, fast_gb)

    shared = _prep_shared(ins)
    in_maps = []
    for i in range(N_CORES):
        m = dict(shared)
        m["x"] = np.ascontiguousarray(x[:, i * BC : (i + 1) * BC, :])
        in_maps.append(m)

    res = run_bass_kernel_spmd(nc, in_maps, core_ids=list(range(N_CORES)))
    out = np.concatenate(
        [np.asarray(res.results[i]["out"]) for i in range(N_CORES)], axis=0
    )
    return out.astype(np.float32)


if __name__ == "__main__":
    rng = np.random.default_rng(0)
    ins = {
        "x": rng.standard_normal((M, B_FULL, E), dtype=np.float32),
        "Wq": (rng.standard_normal((M, E, A)) / np.sqrt(E)).astype(np.float32),
        "bq": (rng.standard_normal((M, A)) / np.sqrt(E)).astype(np.float32),
        "Wk": (rng.standard_normal((M, E, A)) / np.sqrt(E)).astype(np.float32),
        "bk": (rng.standard_normal((M, A)) / np.sqrt(E)).astype(np.float32),
        "v": (rng.standard_normal((M, A)) / np.sqrt(A)).astype(np.float32),
        "Wt": (rng.standard_normal((M, M, E, E)) / np.sqrt(E)).astype(np.float32),
        "bt": (rng.standard_normal((M, M, E)) / np.sqrt(E)).astype(np.float32),
        "gamma": np.ones((M * E,), np.float32),
        "beta": np.zeros((M * E,), np.float32),
    }
    out = kernel(**ins)
    print("out", out.shape, out.dtype)
